# revision 21
# baseline (speedup 1.0000x reference)
"""Causal self-attention (B=8, T=1024, C=768, NH=12) on 8 TRN2 NeuronCores.

Strategy: pure batch data-parallel — core b computes batch element b end to
end (no collectives).

v3 highlights (baseline 266909ns -> v2 248375ns -> this):
  * bf16 x / wqk / wv / wp: halves input DMA and enables FWL weight loads.
  * causal masks are one DVE multiply by an upper-tri 0/1 tile on the exp
    output (no PE mask matmuls).
  * AV runs M=65 bf16 (64 head dims + the ones column that produces the
    softmax denominator in psum row 64).
  * engine assignment tuned for balance: PE matmuls; ACT exps + even-head
    evac; DVE masks, normalize muls, recip, odd-head evac, proj bias;
    GpSimd qkT bias + denominator-chain DMA hops; Sync bulk DMA + outputs.
  * software-pipelined softmax denominator: attn core (QK/exp/mask/AV/evac
    + reshape hop) and the chain tail (recip/spread/normalize) of block g-1
    are interleaved so no engine queue head-of-line-blocks the next block.
  * qkT chunks are emitted two head-pairs ahead so their GpSimd bias ops
    sit in front of the chain DMAs in queue order.

Per-core dataflow (everything kept "transposed", i.e. [feature, time]):
  xT [C, T] bf16                             (host pre-transposes x[b])
  qkT[j, t] = Wqk[:, j].T x                  attT-friendly layout
  v  [t, j] = x Wv                           AV-friendly layout, one ones
                                             column per head (denominator)
  attT[tk, tq] = kT.T @ qT   per head pair   PSUM [128, 2, 512]
  expT = exp(scale * attT)                   no max-sub: |logits| small
  diag blocks: expT *= tri01 (DVE)           causal mask
  out_aug[d|denom] = [v | 1].T @ expT        M=65, psum row 64 = denom
  rawT[j, t] = out_aug[d] * (1/denom)        denom spread via DMA reshape
  yT[e, t] = Wp.T @ rawT + bp                bf16, host transposes back
"""

import os
import sys

import numpy as np

for _p in ("/opt/trn_rl_repo", "/root/.axon_site/_ro/trn_rl_repo"):
    if os.path.isdir(_p) and _p not in sys.path:
        sys.path.insert(0, _p)

import ml_dtypes

import concourse.bacc as bacc
import concourse.mybir as mybir
import concourse.tile as tile
from concourse.bass import ts
from concourse.bass_utils import run_bass_kernel_spmd

B, T, C = 8, 1024, 768
NH, HD = 12, 64
P = 128
NCORES = 8
CC = 6                 # contraction chunks over C
JQK = 12               # output chunks for q|k
EC = 6                 # output chunks for the projection
TQ = 512               # moving-dim tile (max psum bank width)
NTQ = 2
NTK = 8                # key chunks
G = 6                  # head pairs (two 64-wide heads per 128 partitions)
VW = 2 * HD + 2        # 130: per-pair v layout [d_even(64), 1, d_odd(64), 1]
JV = 384               # v output tile width (3 head pairs)
SCALE = 1.0 / float(np.sqrt(HD))
# packed constants tensor layout: [bqk(12) | bp(6) | bvr(768) | tri01(128)]
NCST = JQK + EC + C + P
F32 = mybir.dt.float32
F32R = mybir.dt.float32r
BF16 = mybir.dt.bfloat16
F16 = mybir.dt.float16
AF = mybir.ActivationFunctionType
ADD = mybir.AluOpType.add
MULT = mybir.AluOpType.mult

_CACHE = {}


def _build():
    if "nc" in _CACHE:
        return _CACHE["nc"]

    nc = bacc.Bacc("TRN2", target_bir_lowering=False, debug=False)

    xT = nc.dram_tensor("xT", [C, T], BF16, kind="ExternalInput")
    wqk = nc.dram_tensor("wqk", [C, 2 * C], BF16, kind="ExternalInput")
    wv = nc.dram_tensor("wv", [C, C], BF16, kind="ExternalInput")
    wp = nc.dram_tensor("wp", [C, C], BF16, kind="ExternalInput")
    cst = nc.dram_tensor("cst", [P, NCST], F32, kind="ExternalInput")
    yT = nc.dram_tensor("yT", [C, T], F32, kind="ExternalOutput")

    xT_r = xT[:].rearrange("(h p) t -> p h t", p=P)  # h = 6 chunks
    wqk_r = wqk[:].rearrange("(o p) j -> p o j", p=P)
    wv_r = wv[:].rearrange("(o p) j -> p o j", p=P)
    wp_r = wp[:].rearrange("(o p) e -> p o e", p=P)
    yT_r = yT[:].rearrange("(o p) t -> p o t", p=P)

    with tile.TileContext(nc) as tc:
        with (
            tc.tile_pool(name="const", bufs=1) as constp,
            tc.tile_pool(name="xt", bufs=3) as xtp,
            tc.tile_pool(name="wqk", bufs=4) as wqkp,
            tc.tile_pool(name="wv", bufs=1) as wvp,
            tc.tile_pool(name="qkt", bufs=1) as qkTp,
            tc.tile_pool(name="vaug", bufs=1) as vap,
            tc.tile_pool(name="raw", bufs=1) as rawp,
            tc.tile_pool(name="wp", bufs=6) as wpp,
            tc.tile_pool(name="exp", bufs=3) as expp,
            tc.tile_pool(name="rr", bufs=4) as rrp,
            tc.tile_pool(name="yt", bufs=3) as ytp,
            tc.tile_pool(name="psA", bufs=2, space="PSUM") as psA,
            tc.tile_pool(name="psB", bufs=4, space="PSUM") as psB,
        ):
            # ---- resident tensors -------------------------------------
            # x streams in as 3 two-chunk DMAs so the PE can start early;
            # the first weight chunks slot between them.
            # ~90 tiny matmuls on a dummy tile warm the HAM clock gate
            # (K=4/8 -> 8/8 takes ~3.4us of PE activity) while the input
            # DMAs stream in, so the first real matmuls run at 2.4 GHz
            dumw = constp.tile([1, 16], BF16)
            nc.vector.memset(dumw[:], 0.0)
            dups = psB.tile([16, 16], F32, tag="mm", name="dups")
            for _ in range(320):
                nc.tensor.matmul(
                    dups[0:1, :], dumw[0:1, 0:1], dumw[0:1, :],
                    start=True, stop=True,
                )

            def load_wt(jc):
                wt = wqkp.tile([P, CC, P], BF16, tag="wqk", name="wt")
                nc.sync.dma_start(wt[:], wqk_r[:, :, ts(jc, P)])
                return wt

            wt0 = load_wt(0)
            xt2s = []
            xt_t = xtp.tile([P, 2, T], BF16, tag="xt", name="xt0")
            nc.sync.dma_start(xt_t[:, 0, :], xT_r[:, 0, :])
            nc.sync.dma_start(xt_t[:, 1, :], xT_r[:, 1, :])
            xt2s.append(xt_t)
            wts0 = (wt0, load_wt(G))

            for h in (1, 2):
                xt_t = xtp.tile([P, 2, T], BF16, tag="xt", name=f"xt{h}")
                nc.sync.dma_start(xt_t[:], xT_r[:, 2 * h : 2 * h + 2, :])
                xt2s.append(xt_t)
            xts = [xt2s[cc // 2][:, cc % 2, :] for cc in range(CC)]

            cst_sb = constp.tile([P, NCST], F32)
            nc.sync.dma_start(cst_sb[:], cst[:])
            bqk_sb = cst_sb[:, 0:JQK]
            bp_sb = cst_sb[:, JQK : JQK + EC]
            bv_sb = cst_sb[:, JQK + EC : JQK + EC + C]
            tri_sb = cst_sb[:, JQK + EC + C : NCST]

            qkT_sb = qkTp.tile([P, JQK, T], BF16)
            v_sb = vap.tile([P, NTK, G * VW], BF16)
            v4 = v_sb[:].rearrange("p n (g w) -> p n g w", w=VW)
            rawT = rawp.tile([P, CC, T], BF16)

            # ones columns feed the softmax-denominator trick
            onec = constp.tile([P, 1], F32)
            nc.vector.memset(onec[:], 1.0)
            ones16 = constp.tile([1, 64], F16)
            nc.vector.memset(ones16[:], 1.0)
            ones_src = onec[:, None, None, :].to_broadcast([P, NTK, G, 1])
            nc.vector.tensor_copy(v4[:, :, :, HD : HD + 1], ones_src)
            nc.vector.tensor_copy(v4[:, :, :, VW - 1 : VW], ones_src)

            wv_sb = wvp.tile([P, CC, C], BF16)

            def v_phase():
                for tc_i in range(NTK):
                    for jn in range(C // JV):
                        ps = psB.tile([P, TQ], F32, tag="mm", name="psv")
                        for cc in range(CC):
                            nc.tensor.matmul(
                                ps[:, :JV],
                                xts[cc][:, ts(tc_i, P)],
                                wv_sb[:, cc, ts(jn, JV)],
                                start=(cc == 0),
                                stop=(cc == CC - 1),
                            )
                        g0 = jn * (JV // P)  # 3 head pairs per 384 cols
                        srcv = ps[:, :JV].rearrange(
                            "p (g h d) -> p g h d", h=2, d=HD
                        )
                        bias = bv_sb[:, ts(jn, JV)].rearrange(
                            "p (g h d) -> p g h d", h=2, d=HD
                        )
                        nc.vector.tensor_tensor(
                            v4[:, tc_i, g0 : g0 + 3, 0:HD],
                            srcv[:, :, 0, :],
                            bias[:, :, 0, :],
                            ADD,
                        )
                        nc.vector.tensor_tensor(
                            v4[:, tc_i, g0 : g0 + 3, HD + 1 : VW - 1],
                            srcv[:, :, 1, :],
                            bias[:, :, 1, :],
                            ADD,
                        )

            def qkt_chunk(jc, wt):
                for t2 in range(NTQ):
                    ps = psB.tile([P, TQ], F32, tag="mm", name="psq")
                    for cc in range(CC):
                        nc.tensor.matmul(
                            ps[:],
                            wt[:, cc, :],
                            xts[cc][:, ts(t2, TQ)],
                            start=(cc == 0),
                            stop=(cc == CC - 1),
                        )
                    nc.vector.tensor_scalar_add(
                        qkT_sb[:, jc, ts(t2, TQ)],
                        ps[:],
                        bqk_sb[:, jc : jc + 1],
                    )

            # later qkT chunks are emitted as "filler" closures, consumed
            # one matmul at a time between attention AV pairs: the PE then
            # has independent work whenever it would stall on an exp
            fillq = []

            def qkt_fillers(jc, wt):
                for t2 in range(NTQ):
                    state = {}

                    def mk(cc, t2=t2, state=state):
                        def run():
                            if cc == 0:
                                state["ps"] = psB.tile(
                                    [P, TQ], F32, tag="mm", name="psq"
                                )
                            ps = state["ps"]
                            nc.tensor.matmul(
                                ps[:],
                                wt[:, cc, :],
                                xts[cc][:, ts(t2, TQ)],
                                start=(cc == 0),
                                stop=(cc == CC - 1),
                            )
                            if cc == CC - 1:
                                nc.vector.tensor_scalar_add(
                                    qkT_sb[:, jc, ts(t2, TQ)],
                                    ps[:],
                                    bqk_sb[:, jc : jc + 1],
                                )

                        return run

                    fillq.extend(mk(cc) for cc in range(CC))

            def pop_fill(k):
                for _ in range(k):
                    if fillq:
                        fillq.pop(0)()

            fill0 = nc.gpsimd.to_reg(0.0)

            def attn_core(g, t2):
                """QK -> exp -> mask -> AV -> evac + denominator reshape.

                Returns state for attn_finish (the chain tail), which is
                emitted one block later so its DVE/GpSimd waits never
                head-of-line-block this block's mask/exp traffic. The AV
                pair for tile tkc is emitted after QK of tkc+1 so the PE
                has work while the mask op runs.
                """
                jq, jk = g, G + g
                hi = 4 * (t2 + 1)  # causal: key chunks 0..hi-1
                avs = []
                for par in (0, 1):
                    av = psB.tile([P, TQ], F32, tag="mm", name=f"av{par}")
                    avs.append(av)

                def av_pair(e, cs, tkc):
                    for par in (0, 1):
                        vlo = g * VW + (HD + 1) * par
                        nc.tensor.matmul(
                            avs[par][0:65, cs:],
                            v_sb[:, tkc, vlo : vlo + HD + 1],
                            e[:, par, cs:],
                            start=(tkc == 0),
                            stop=(tkc == hi - 1),
                        )

                prev = None
                for tkc in range(hi):
                    csr = tkc * P - t2 * TQ  # diag block start col
                    cs = max(0, csr)
                    pa = psA.tile([P, 2, TQ], F32, tag="pa", name="pa")
                    for par in (0, 1):
                        qrow = HD * par
                        nc.tensor.matmul(
                            pa[:, par, cs:TQ],
                            qkT_sb[qrow : qrow + HD, jk, ts(tkc, P)],
                            qkT_sb[
                                qrow : qrow + HD,
                                jq,
                                t2 * TQ + cs : (t2 + 1) * TQ,
                            ],
                            start=True,
                            stop=True,
                        )
                    e = expp.tile([P, 2, TQ], BF16, tag="exp", name="e")
                    nc.scalar.activation(
                        e[:, :, cs:], pa[:, :, cs:], AF.Exp, scale=SCALE
                    )
                    if csr >= 0:
                        # causal mask on the diagonal 128-wide block: keep
                        # e[tk, tq] only where tq_local >= tk_local
                        # (iota = tq_local - partition, keep iota >= 0)
                        nc.gpsimd.affine_select(
                            e[:, :, cs : cs + P],
                            e[:, :, cs : cs + P],
                            pattern=[[0, 2], [1, P]],
                            compare_op=mybir.AluOpType.is_ge,
                            fill=fill0,
                            base=0,
                            channel_multiplier=-1,
                        )
                    if prev is not None:
                        av_pair(*prev)
                        pop_fill(2)
                    prev = (e, cs, tkc)
                av_pair(*prev)
                pop_fill(2)
                # evacuate out_aug [head dims | denominator] to SBUF right
                # away (frees the psum banks). Then one DMA reshapes both
                # denominator rows across all 128 lanes for a cheap
                # reciprocal.
                asb = rrp.tile([65, 2, TQ], F32, tag="avsb", name="asb")
                nc.vector.tensor_scalar_add(asb[:, 0, :], avs[0][0:65, :], 0.0)
                nc.vector.tensor_scalar_add(asb[:, 1, :], avs[1][0:65, :], 0.0)
                rd = rrp.tile([P, 8], F32, tag="rd", name="rd")
                nc.gpsimd.dma_start(rd[:], asb[64:65, :, :])
                return g, t2, asb, rd

            def attn_finish(state):
                """recip -> spread -> normalize; emitted two blocks late.

                The reciprocals spread across 64 partitions via a K=1
                fp16 PE matmul (ones[1,64].T @ recip_row) — a broadcast
                DMA would re-read one SBUF partition 64 times and take
                ~7us; the matmul takes ~430ns of PE.
                """
                g, t2, asb, rd = state
                rd2 = rrp.tile([P, 8], F16, tag="rd2", name="rd2")
                with nc.allow_low_precision(
                    reason="fp16 softmax denominators keep 11 mantissa bits"
                ):
                    nc.vector.reciprocal(rd2[:], rd[:])
                rro = rrp.tile([1, 2 * TQ], F16, tag="rro", name="rro")
                nc.gpsimd.dma_start(rro[0:1, :], rd2[:])
                prs = []
                for par in (0, 1):
                    pr = psB.tile([64, TQ], F32, tag="mm", name=f"pr{par}")
                    nc.tensor.matmul(
                        pr[:],
                        ones16[:],
                        rro[0:1, ts(par, TQ)],
                        start=True,
                        stop=True,
                    )
                    prs.append(pr)
                nc.vector.tensor_mul(
                    rawT[0:64, g, ts(t2, TQ)], asb[0:64, 0, :], prs[0][:]
                )
                tmp = rrp.tile([64, TQ], BF16, tag="otmp", name="tmp")
                nc.vector.tensor_mul(tmp[:], asb[0:64, 1, :], prs[1][:])
                nc.sync.dma_start(rawT[64:128, g, ts(t2, TQ)], tmp[:])

            def load_wpt(ec):
                wpt = wpp.tile([P, CC, P], BF16, tag="wp", name="wpt")
                nc.sync.dma_start(wpt[:], wp_r[:, :, ts(ec, P)])
                return wpt

            def proj_half(t2, wpts):
                for ec in range(EC):
                    wpt = wpts[ec]
                    ps = psB.tile([P, TQ], F32, tag="mm", name="psp_")
                    for jc in range(CC):
                        nc.tensor.matmul(
                            ps[:],
                            wpt[:, jc, :],
                            rawT[:, jc, ts(t2, TQ)],
                            start=(jc == 0),
                            stop=(jc == CC - 1),
                        )
                    yt = ytp.tile([P, TQ], F32, tag="yt", name="yt")
                    nc.vector.tensor_scalar_add(
                        yt[:], ps[:], bp_sb[:, ec : ec + 1]
                    )
                    nc.sync.dma_start(yT_r[:, ec, ts(t2, TQ)], yt[:])

            # emission order: qkT chunks run two head-pairs ahead of the
            # attention sweep; the two t2 halves of each head pair run
            # back to back (the heavy t2=1 exp/AV smooths engine load);
            # each block's chain tail lands one block late; proj_half(0)
            # covers the last chain before proj_half(1) needs it.
            qkt_chunk(0, wts0[0])
            qkt_chunk(G, wts0[1])
            nc.sync.dma_start(wv_sb[:, 0:3, :], wv_r[:, 0:3, :])
            nc.sync.dma_start(wv_sb[:, 3:6, :], wv_r[:, 3:6, :])
            v_phase()
            wts1 = (load_wt(1), load_wt(G + 1))
            qkt_chunk(1, wts1[0])
            qkt_chunk(G + 1, wts1[1])
            # chain tails run two blocks late: the ~6us DMA-latency of the
            # denominator chain is then fully covered by other blocks, so
            # the DVE queue never stalls on an rr spread in flight
            from collections import deque

            pend = deque()

            def push_core(st):
                pend.append(st)
                if len(pend) > 2:
                    attn_finish(pend.popleft())

            for g in range(G):
                if g + 2 < G:
                    wts = (load_wt(g + 2), load_wt(G + g + 2))
                    qkt_fillers(g + 2, wts[0])
                    qkt_fillers(G + g + 2, wts[1])
                elif g == G - 2:
                    wpts = [load_wpt(ec) for ec in range(EC)]
                push_core(attn_core(g, 0))
                push_core(attn_core(g, 1))
            attn_finish(pend.popleft())  # (5, 0)
            proj_half(0, wpts)
            attn_finish(pend.popleft())  # (5, 1)
            proj_half(1, wpts)

    nc.compile()
    _CACHE["nc"] = nc
    return nc


def make_in_maps(x, w_attn, b_attn, w_proj, b_proj):
    x = np.ascontiguousarray(np.asarray(x, dtype=np.float32))
    w_attn = np.ascontiguousarray(np.asarray(w_attn, dtype=np.float32))
    b_attn = np.ascontiguousarray(np.asarray(b_attn, dtype=np.float32))
    w_proj = np.ascontiguousarray(np.asarray(w_proj, dtype=np.float32))
    b_proj = np.ascontiguousarray(np.asarray(b_proj, dtype=np.float32))

    bf = ml_dtypes.bfloat16
    wqk = np.ascontiguousarray(w_attn[:, : 2 * C].astype(bf))
    wv = np.ascontiguousarray(w_attn[:, 2 * C :].astype(bf))
    wpb = np.ascontiguousarray(w_proj.astype(bf))
    cstm = np.zeros((P, NCST), dtype=np.float32)
    cstm[:, 0:JQK] = b_attn[: 2 * C].reshape(JQK, P).T
    cstm[:, JQK : JQK + EC] = b_proj.reshape(EC, P).T
    cstm[:, JQK + EC : JQK + EC + C] = np.tile(
        b_attn[2 * C :][None, :], (P, 1)
    )
    # tri01[p, c] = 1 where c >= p (query local index >= key local index)
    cstm[:, JQK + EC + C : NCST] = (
        np.arange(P)[None, :] >= np.arange(P)[:, None]
    ).astype(np.float32)

    shared = {"wqk": wqk, "wv": wv, "wp": wpb, "cst": cstm}
    return [
        {"xT": np.ascontiguousarray(x[b].T.astype(bf)), **shared}
        for b in range(NCORES)
    ]


def kernel(**inputs):
    nc = _build()
    in_maps = make_in_maps(
        inputs["x"],
        inputs["w_attn"],
        inputs["b_attn"],
        inputs["w_proj"],
        inputs["b_proj"],
    )
    res = run_bass_kernel_spmd(nc, in_maps, list(range(NCORES)))
    out = np.stack(
        [np.ascontiguousarray(res.results[b]["yT"].T) for b in range(NCORES)]
    )
    return out.astype(np.float32)


# revision 22
# speedup vs baseline: 1.0258x; 1.0258x over previous
"""Causal self-attention (B=8, T=1024, C=768, NH=12) on 8 TRN2 NeuronCores.

Strategy: pure batch data-parallel — core b computes batch element b end to
end (no collectives).

v3 highlights (baseline 266909ns -> v2 248375ns -> this):
  * bf16 x / wqk / wv / wp: halves input DMA and enables FWL weight loads.
  * causal masks are one DVE multiply by an upper-tri 0/1 tile on the exp
    output (no PE mask matmuls).
  * AV runs M=65 bf16 (64 head dims + the ones column that produces the
    softmax denominator in psum row 64).
  * engine assignment tuned for balance: PE matmuls; ACT exps + even-head
    evac; DVE masks, normalize muls, recip, odd-head evac, proj bias;
    GpSimd qkT bias + denominator-chain DMA hops; Sync bulk DMA + outputs.
  * software-pipelined softmax denominator: attn core (QK/exp/mask/AV/evac
    + reshape hop) and the chain tail (recip/spread/normalize) of block g-1
    are interleaved so no engine queue head-of-line-blocks the next block.
  * qkT chunks are emitted two head-pairs ahead so their GpSimd bias ops
    sit in front of the chain DMAs in queue order.

Per-core dataflow (everything kept "transposed", i.e. [feature, time]):
  xT [C, T] bf16                             (host pre-transposes x[b])
  qkT[j, t] = Wqk[:, j].T x                  attT-friendly layout
  v  [t, j] = x Wv                           AV-friendly layout, one ones
                                             column per head (denominator)
  attT[tk, tq] = kT.T @ qT   per head pair   PSUM [128, 2, 512]
  expT = exp(scale * attT)                   no max-sub: |logits| small
  diag blocks: expT *= tri01 (DVE)           causal mask
  out_aug[d|denom] = [v | 1].T @ expT        M=65, psum row 64 = denom
  rawT[j, t] = out_aug[d] * (1/denom)        denom spread via DMA reshape
  yT[e, t] = Wp.T @ rawT + bp                bf16, host transposes back
"""

import os
import sys

import numpy as np

for _p in ("/opt/trn_rl_repo", "/root/.axon_site/_ro/trn_rl_repo"):
    if os.path.isdir(_p) and _p not in sys.path:
        sys.path.insert(0, _p)

import ml_dtypes

import concourse.bacc as bacc
import concourse.mybir as mybir
import concourse.tile as tile
from concourse.bass import ts
from concourse.bass_utils import run_bass_kernel_spmd

B, T, C = 8, 1024, 768
NH, HD = 12, 64
P = 128
NCORES = 8
CC = 6                 # contraction chunks over C
JQK = 12               # output chunks for q|k
EC = 6                 # output chunks for the projection
TQ = 512               # moving-dim tile (max psum bank width)
NTQ = 2
NTK = 8                # key chunks
G = 6                  # head pairs (two 64-wide heads per 128 partitions)
VW = 2 * HD + 2        # 130: per-pair v layout [d_even(64), 1, d_odd(64), 1]
JV = 384               # v output tile width (3 head pairs)
SCALE = 1.0 / float(np.sqrt(HD))
# packed constants tensor layout: [bqk(12) | bp(6) | bvr(768) | tri01(128)]
NCST = JQK + EC + C + P
F32 = mybir.dt.float32
F32R = mybir.dt.float32r
BF16 = mybir.dt.bfloat16
F16 = mybir.dt.float16
AF = mybir.ActivationFunctionType
ADD = mybir.AluOpType.add
MULT = mybir.AluOpType.mult

_CACHE = {}


def _build():
    if "nc" in _CACHE:
        return _CACHE["nc"]

    nc = bacc.Bacc("TRN2", target_bir_lowering=False, debug=False)

    xT = nc.dram_tensor("xT", [C, T], BF16, kind="ExternalInput")
    wqk = nc.dram_tensor("wqk", [C, 2 * C], BF16, kind="ExternalInput")
    wv = nc.dram_tensor("wv", [C, C], BF16, kind="ExternalInput")
    wp = nc.dram_tensor("wp", [C, C], BF16, kind="ExternalInput")
    cst = nc.dram_tensor("cst", [P, NCST], F32, kind="ExternalInput")
    yT = nc.dram_tensor("yT", [C, T], F32, kind="ExternalOutput")

    xT_r = xT[:].rearrange("(h p) t -> p h t", p=P)  # h = 6 chunks
    wqk_r = wqk[:].rearrange("(o p) j -> p o j", p=P)
    wv_r = wv[:].rearrange("(o p) j -> p o j", p=P)
    wp_r = wp[:].rearrange("(o p) e -> p o e", p=P)
    yT_r = yT[:].rearrange("(o p) t -> p o t", p=P)

    with tile.TileContext(nc) as tc:
        with (
            tc.tile_pool(name="const", bufs=1) as constp,
            tc.tile_pool(name="xt", bufs=3) as xtp,
            tc.tile_pool(name="wqk", bufs=4) as wqkp,
            tc.tile_pool(name="wv", bufs=1) as wvp,
            tc.tile_pool(name="qkt", bufs=1) as qkTp,
            tc.tile_pool(name="vaug", bufs=1) as vap,
            tc.tile_pool(name="raw", bufs=1) as rawp,
            tc.tile_pool(name="wp", bufs=6) as wpp,
            tc.tile_pool(name="exp", bufs=3) as expp,
            tc.tile_pool(name="rr", bufs=4) as rrp,
            tc.tile_pool(name="yt", bufs=3) as ytp,
            tc.tile_pool(name="psA", bufs=2, space="PSUM") as psA,
            tc.tile_pool(name="psB", bufs=4, space="PSUM") as psB,
        ):
            # ---- resident tensors -------------------------------------
            # x streams in as 3 two-chunk DMAs so the PE can start early;
            # the first weight chunks slot between them.
            # ~90 tiny matmuls on a dummy tile warm the HAM clock gate
            # (K=4/8 -> 8/8 takes ~3.4us of PE activity) while the input
            # DMAs stream in, so the first real matmuls run at 2.4 GHz
            dumw = constp.tile([1, TQ], BF16)
            nc.vector.memset(dumw[:], 0.0)
            dups = psB.tile([1, TQ], F32, tag="mm", name="dups")
            for _ in range(12):
                nc.tensor.matmul(
                    dups[0:1, :], dumw[0:1, 0:1], dumw[0:1, :],
                    start=True, stop=True,
                )

            def load_wt(jc):
                wt = wqkp.tile([P, CC, P], BF16, tag="wqk", name="wt")
                nc.sync.dma_start(wt[:], wqk_r[:, :, ts(jc, P)])
                return wt

            wt0 = load_wt(0)
            xt2s = []
            xt_t = xtp.tile([P, 2, T], BF16, tag="xt", name="xt0")
            nc.sync.dma_start(xt_t[:, 0, :], xT_r[:, 0, :])
            nc.sync.dma_start(xt_t[:, 1, :], xT_r[:, 1, :])
            xt2s.append(xt_t)
            wts0 = (wt0, load_wt(G))

            for h in (1, 2):
                xt_t = xtp.tile([P, 2, T], BF16, tag="xt", name=f"xt{h}")
                nc.sync.dma_start(xt_t[:], xT_r[:, 2 * h : 2 * h + 2, :])
                xt2s.append(xt_t)
            xts = [xt2s[cc // 2][:, cc % 2, :] for cc in range(CC)]

            cst_sb = constp.tile([P, NCST], F32)
            nc.sync.dma_start(cst_sb[:], cst[:])
            bqk_sb = cst_sb[:, 0:JQK]
            bp_sb = cst_sb[:, JQK : JQK + EC]
            bv_sb = cst_sb[:, JQK + EC : JQK + EC + C]
            tri_sb = cst_sb[:, JQK + EC + C : NCST]

            qkT_sb = qkTp.tile([P, JQK, T], BF16)
            v_sb = vap.tile([P, NTK, G * VW], BF16)
            v4 = v_sb[:].rearrange("p n (g w) -> p n g w", w=VW)
            rawT = rawp.tile([P, CC, T], BF16)

            # ones columns feed the softmax-denominator trick
            onec = constp.tile([P, 1], F32)
            nc.vector.memset(onec[:], 1.0)
            ones16 = constp.tile([1, 64], F16)
            nc.vector.memset(ones16[:], 1.0)
            ones_src = onec[:, None, None, :].to_broadcast([P, NTK, G, 1])
            nc.vector.tensor_copy(v4[:, :, :, HD : HD + 1], ones_src)
            nc.vector.tensor_copy(v4[:, :, :, VW - 1 : VW], ones_src)

            wv_sb = wvp.tile([P, CC, C], BF16)

            def v_phase():
                for tc_i in range(NTK):
                    for jn in range(C // JV):
                        ps = psB.tile([P, TQ], F32, tag="mm", name="psv")
                        for cc in range(CC):
                            nc.tensor.matmul(
                                ps[:, :JV],
                                xts[cc][:, ts(tc_i, P)],
                                wv_sb[:, cc, ts(jn, JV)],
                                start=(cc == 0),
                                stop=(cc == CC - 1),
                            )
                        g0 = jn * (JV // P)  # 3 head pairs per 384 cols
                        srcv = ps[:, :JV].rearrange(
                            "p (g h d) -> p g h d", h=2, d=HD
                        )
                        bias = bv_sb[:, ts(jn, JV)].rearrange(
                            "p (g h d) -> p g h d", h=2, d=HD
                        )
                        nc.vector.tensor_tensor(
                            v4[:, tc_i, g0 : g0 + 3, 0:HD],
                            srcv[:, :, 0, :],
                            bias[:, :, 0, :],
                            ADD,
                        )
                        nc.vector.tensor_tensor(
                            v4[:, tc_i, g0 : g0 + 3, HD + 1 : VW - 1],
                            srcv[:, :, 1, :],
                            bias[:, :, 1, :],
                            ADD,
                        )

            def qkt_chunk(jc, wt):
                for t2 in range(NTQ):
                    ps = psB.tile([P, TQ], F32, tag="mm", name="psq")
                    for cc in range(CC):
                        nc.tensor.matmul(
                            ps[:],
                            wt[:, cc, :],
                            xts[cc][:, ts(t2, TQ)],
                            start=(cc == 0),
                            stop=(cc == CC - 1),
                        )
                    nc.vector.tensor_scalar_add(
                        qkT_sb[:, jc, ts(t2, TQ)],
                        ps[:],
                        bqk_sb[:, jc : jc + 1],
                    )

            # later qkT chunks are emitted as "filler" closures, consumed
            # one matmul at a time between attention AV pairs: the PE then
            # has independent work whenever it would stall on an exp
            fillq = []

            def qkt_fillers(jc, wt):
                for t2 in range(NTQ):
                    state = {}

                    def mk(cc, t2=t2, state=state):
                        def run():
                            if cc == 0:
                                state["ps"] = psB.tile(
                                    [P, TQ], F32, tag="mm", name="psq"
                                )
                            ps = state["ps"]
                            nc.tensor.matmul(
                                ps[:],
                                wt[:, cc, :],
                                xts[cc][:, ts(t2, TQ)],
                                start=(cc == 0),
                                stop=(cc == CC - 1),
                            )
                            if cc == CC - 1:
                                nc.vector.tensor_scalar_add(
                                    qkT_sb[:, jc, ts(t2, TQ)],
                                    ps[:],
                                    bqk_sb[:, jc : jc + 1],
                                )

                        return run

                    fillq.extend(mk(cc) for cc in range(CC))

            def pop_fill(k):
                for _ in range(k):
                    if fillq:
                        fillq.pop(0)()

            fill0 = nc.gpsimd.to_reg(0.0)

            def attn_core(g, t2):
                """QK -> exp -> mask -> AV -> evac + denominator reshape.

                Returns state for attn_finish (the chain tail), which is
                emitted one block later so its DVE/GpSimd waits never
                head-of-line-block this block's mask/exp traffic. The AV
                pair for tile tkc is emitted after QK of tkc+1 so the PE
                has work while the mask op runs.
                """
                jq, jk = g, G + g
                hi = 4 * (t2 + 1)  # causal: key chunks 0..hi-1
                avs = []
                for par in (0, 1):
                    av = psB.tile([P, TQ], F32, tag="mm", name=f"av{par}")
                    avs.append(av)

                def av_pair(e, cs, tkc):
                    for par in (0, 1):
                        vlo = g * VW + (HD + 1) * par
                        nc.tensor.matmul(
                            avs[par][0:65, cs:],
                            v_sb[:, tkc, vlo : vlo + HD + 1],
                            e[:, par, cs:],
                            start=(tkc == 0),
                            stop=(tkc == hi - 1),
                        )

                prev = None
                for tkc in range(hi):
                    csr = tkc * P - t2 * TQ  # diag block start col
                    cs = max(0, csr)
                    pa = psA.tile([P, 2, TQ], F32, tag="pa", name="pa")
                    for par in (0, 1):
                        qrow = HD * par
                        nc.tensor.matmul(
                            pa[:, par, cs:TQ],
                            qkT_sb[qrow : qrow + HD, jk, ts(tkc, P)],
                            qkT_sb[
                                qrow : qrow + HD,
                                jq,
                                t2 * TQ + cs : (t2 + 1) * TQ,
                            ],
                            start=True,
                            stop=True,
                        )
                    e = expp.tile([P, 2, TQ], BF16, tag="exp", name="e")
                    nc.scalar.activation(
                        e[:, :, cs:], pa[:, :, cs:], AF.Exp, scale=SCALE
                    )
                    if csr >= 0:
                        # causal mask on the diagonal 128-wide block: keep
                        # e[tk, tq] only where tq_local >= tk_local
                        # (iota = tq_local - partition, keep iota >= 0)
                        nc.gpsimd.affine_select(
                            e[:, :, cs : cs + P],
                            e[:, :, cs : cs + P],
                            pattern=[[0, 2], [1, P]],
                            compare_op=mybir.AluOpType.is_ge,
                            fill=fill0,
                            base=0,
                            channel_multiplier=-1,
                        )
                    if prev is not None:
                        av_pair(*prev)
                        pop_fill(2)
                    prev = (e, cs, tkc)
                av_pair(*prev)
                pop_fill(2)
                # evacuate out_aug [head dims | denominator] to SBUF right
                # away (frees the psum banks). Then one DMA reshapes both
                # denominator rows across all 128 lanes for a cheap
                # reciprocal.
                asb = rrp.tile([65, 2, TQ], F32, tag="avsb", name="asb")
                nc.vector.tensor_scalar_add(asb[:, 0, :], avs[0][0:65, :], 0.0)
                nc.vector.tensor_scalar_add(asb[:, 1, :], avs[1][0:65, :], 0.0)
                rd = rrp.tile([P, 8], F32, tag="rd", name="rd")
                nc.gpsimd.dma_start(rd[:], asb[64:65, :, :])
                return g, t2, asb, rd

            def attn_finish(state):
                """recip -> spread -> normalize; emitted two blocks late.

                The reciprocals spread across 64 partitions via a K=1
                fp16 PE matmul (ones[1,64].T @ recip_row) — a broadcast
                DMA would re-read one SBUF partition 64 times and take
                ~7us; the matmul takes ~430ns of PE.
                """
                g, t2, asb, rd = state
                rd2 = rrp.tile([P, 8], F16, tag="rd2", name="rd2")
                with nc.allow_low_precision(
                    reason="fp16 softmax denominators keep 11 mantissa bits"
                ):
                    nc.vector.reciprocal(rd2[:], rd[:])
                rro = rrp.tile([1, 2 * TQ], F16, tag="rro", name="rro")
                nc.gpsimd.dma_start(rro[0:1, :], rd2[:])
                prs = []
                for par in (0, 1):
                    pr = psB.tile([64, TQ], F32, tag="mm", name=f"pr{par}")
                    nc.tensor.matmul(
                        pr[:],
                        ones16[:],
                        rro[0:1, ts(par, TQ)],
                        start=True,
                        stop=True,
                    )
                    prs.append(pr)
                nc.vector.tensor_mul(
                    rawT[0:64, g, ts(t2, TQ)], asb[0:64, 0, :], prs[0][:]
                )
                tmp = rrp.tile([64, TQ], BF16, tag="otmp", name="tmp")
                nc.vector.tensor_mul(tmp[:], asb[0:64, 1, :], prs[1][:])
                nc.sync.dma_start(rawT[64:128, g, ts(t2, TQ)], tmp[:])

            def load_wpt(ec):
                wpt = wpp.tile([P, CC, P], BF16, tag="wp", name="wpt")
                nc.sync.dma_start(wpt[:], wp_r[:, :, ts(ec, P)])
                return wpt

            def proj_half(t2, wpts):
                for ec in range(EC):
                    wpt = wpts[ec]
                    ps = psB.tile([P, TQ], F32, tag="mm", name="psp_")
                    for jc in range(CC):
                        nc.tensor.matmul(
                            ps[:],
                            wpt[:, jc, :],
                            rawT[:, jc, ts(t2, TQ)],
                            start=(jc == 0),
                            stop=(jc == CC - 1),
                        )
                    yt = ytp.tile([P, TQ], F32, tag="yt", name="yt")
                    nc.vector.tensor_scalar_add(
                        yt[:], ps[:], bp_sb[:, ec : ec + 1]
                    )
                    nc.sync.dma_start(yT_r[:, ec, ts(t2, TQ)], yt[:])

            # emission order: qkT chunks run two head-pairs ahead of the
            # attention sweep; the two t2 halves of each head pair run
            # back to back (the heavy t2=1 exp/AV smooths engine load);
            # each block's chain tail lands one block late; proj_half(0)
            # covers the last chain before proj_half(1) needs it.
            qkt_chunk(0, wts0[0])
            qkt_chunk(G, wts0[1])
            nc.sync.dma_start(wv_sb[:, 0:3, :], wv_r[:, 0:3, :])
            nc.sync.dma_start(wv_sb[:, 3:6, :], wv_r[:, 3:6, :])
            v_phase()
            wts1 = (load_wt(1), load_wt(G + 1))
            qkt_chunk(1, wts1[0])
            qkt_chunk(G + 1, wts1[1])
            # chain tails run two blocks late: the ~6us DMA-latency of the
            # denominator chain is then fully covered by other blocks, so
            # the DVE queue never stalls on an rr spread in flight
            from collections import deque

            pend = deque()

            def push_core(st):
                pend.append(st)
                if len(pend) > 2:
                    attn_finish(pend.popleft())

            for g in range(G):
                if g + 2 < G:
                    wts = (load_wt(g + 2), load_wt(G + g + 2))
                    qkt_fillers(g + 2, wts[0])
                    qkt_fillers(G + g + 2, wts[1])
                elif g == G - 2:
                    wpts = [load_wpt(ec) for ec in range(EC)]
                push_core(attn_core(g, 0))
                push_core(attn_core(g, 1))
            attn_finish(pend.popleft())  # (5, 0)
            proj_half(0, wpts)
            attn_finish(pend.popleft())  # (5, 1)
            proj_half(1, wpts)

    nc.compile()
    _CACHE["nc"] = nc
    return nc


def make_in_maps(x, w_attn, b_attn, w_proj, b_proj):
    x = np.ascontiguousarray(np.asarray(x, dtype=np.float32))
    w_attn = np.ascontiguousarray(np.asarray(w_attn, dtype=np.float32))
    b_attn = np.ascontiguousarray(np.asarray(b_attn, dtype=np.float32))
    w_proj = np.ascontiguousarray(np.asarray(w_proj, dtype=np.float32))
    b_proj = np.ascontiguousarray(np.asarray(b_proj, dtype=np.float32))

    bf = ml_dtypes.bfloat16
    wqk = np.ascontiguousarray(w_attn[:, : 2 * C].astype(bf))
    wv = np.ascontiguousarray(w_attn[:, 2 * C :].astype(bf))
    wpb = np.ascontiguousarray(w_proj.astype(bf))
    cstm = np.zeros((P, NCST), dtype=np.float32)
    cstm[:, 0:JQK] = b_attn[: 2 * C].reshape(JQK, P).T
    cstm[:, JQK : JQK + EC] = b_proj.reshape(EC, P).T
    cstm[:, JQK + EC : JQK + EC + C] = np.tile(
        b_attn[2 * C :][None, :], (P, 1)
    )
    # tri01[p, c] = 1 where c >= p (query local index >= key local index)
    cstm[:, JQK + EC + C : NCST] = (
        np.arange(P)[None, :] >= np.arange(P)[:, None]
    ).astype(np.float32)

    shared = {"wqk": wqk, "wv": wv, "wp": wpb, "cst": cstm}
    return [
        {"xT": np.ascontiguousarray(x[b].T.astype(bf)), **shared}
        for b in range(NCORES)
    ]


def kernel(**inputs):
    nc = _build()
    in_maps = make_in_maps(
        inputs["x"],
        inputs["w_attn"],
        inputs["b_attn"],
        inputs["w_proj"],
        inputs["b_proj"],
    )
    res = run_bass_kernel_spmd(nc, in_maps, list(range(NCORES)))
    out = np.stack(
        [np.ascontiguousarray(res.results[b]["yT"].T) for b in range(NCORES)]
    )
    return out.astype(np.float32)
